# revision 4
# baseline (speedup 1.0000x reference)
"""CAM-module kernel for Trainium2, data-parallel over batch on 8 NeuronCores.

Per core (one batch sample, q = x[b] viewed as (C=512, N=4096) fp32):
  energy   = q @ q^T                      (C, C)   fp8 matmul, fp32 accum
  att[c,d] = exp(m_c - e[c,d]) / Z_c      with m_c = row min of energy
  out      = gamma * (att @ q) + x

The row-max shift of the reference softmax cancels algebraically; only the
row minimum is needed for numerical stability (arguments of exp stay <= 0).

Key structure (v2 — XBAR-transpose restructure):
 - q^T is produced by the DMA XBAR transpose engine (dma_start_transpose),
   viewing adjacent fp8 pairs as fp16.  This removes all 128 identity-matmul
   PE transposes and the 32 DVE PSUM->SBUF copies of the previous version.
   The XBAR output interleaves the two fp8 values of each pair along the
   free dim (A0 B0 A1 B1 ...), which is exactly the DoubleRowSwInterleave
   weight layout (A/B pairs per column, columns reversed), so the energy
   matmuls consume it directly:
     * lhsT = raw interleaved bytes of one 128-c block -> output rows come
       out REVERSED within each 128 block (c_local = 127 - p).
     * rhs  = byte-strided [p, r, (ci c)] view of the same tiles.
 - All row-wise softmax steps run unchanged on the reversed rows.  The
   reversal is undone for free in the attT transposes by using a FLIP
   (anti-identity) matrix instead of the identity as the moving operand;
   the per-row scale gamma/Z is un-reversed by one tiny fp32 matmul
   against a fp32 flip matrix (b2f = J @ bias2).
 - x loads are issued in-tile from the GPSIMD queue in consumption order
   (GPSIMD is otherwise idle in phase A); ACT and DVE each cast one half
   of every chunk.  The tile framework tracks DMA->cast deps (no manual
   semaphores).
 - Phase C: att@q DoubleRow matmuls into [P,1024] 2-bank PSUM groups; the
   epilogue out = po*(gamma/Z) + x is drained by 1024-col ops balanced
   across DVE (scalar_tensor_tensor) and ACT-mul + GPSIMD-add pairs, fp16
   store in 8 x 1MB-ish DMAs.
 - PSUM: 4 banks energy accumulators (reused in phase C for the attT
   staging and the bias2 flip), 4 banks for the [P,1024] att@q groups.
"""

import numpy as np

import concourse.bass as bass
import concourse.tile as tile
from concourse import mybir
from concourse.vector_clock import ScopedClock

P = 128
C = 512
N = 4096
B = 8
CT = C // P   # 4 c-tiles
GG = 4        # 1024-col load groups
UT = N // 256  # 16 uchunks (256 n-values each)

STRIP_TAIL = True

FP32 = mybir.dt.float32
FP16 = mybir.dt.float16
FP8 = mybir.dt.float8e4
DR = mybir.MatmulPerfMode.DoubleRow
DRI = mybir.MatmulPerfMode.DoubleRowSwInterleave
MIN = mybir.AluOpType.min


def _drain_and_barrier_split(self, tick_clock, wait_clock):
    # The pinned walrus rejects >1 sync-wait on TPB_CTRL (Drain); spread the
    # final global-clock waits across a chain of drains, one wait each.
    nc = self.nc
    drain_inst = nc.sync.drain()
    wait_clock.add_sem_waits(
        drain_inst.ins, ScopedClock({None: tick_clock.global_clock})
    )
    si = drain_inst.ins.sync_info
    if si is not None and si.on_wait is not None and len(si.on_wait) > 1:
        waits = list(si.on_wait)
        si.on_wait = waits[:1]
        for w in waits[1:]:
            extra = nc.sync.drain()
            extra.ins.sync_info = mybir.SyncInfo(on_wait=[w], on_update=[])
    nc.all_engine_barrier()
    assert self.sems is not None
    popped = nc._tile_sem_poison_stack.pop()
    assert popped is self._sem_poison
    if not STRIP_TAIL:
        nc.clear_and_free_semaphores(list(self.sems.allocated().values()))
        nc.all_engine_barrier()


tile.TileContext._drain_and_barrier = _drain_and_barrier_split


def _legalize_sync_waits(nc):
    # This walrus build rejects instructions carrying more than one sync-wait.
    # Hoist extra waits onto same-engine NoOps placed immediately before the
    # instruction (engine streams preserve relative order within a block).
    for f in nc.m.functions:
        for bb in f.blocks:
            new = []
            for inst in bb.instructions:
                si = inst.sync_info
                if si is not None and si.on_wait and len(si.on_wait) > 1:
                    waits = list(si.on_wait)
                    for w in waits[:-1]:
                        nop = mybir.InstNoOp(
                            name=nc.get_next_instruction_name(),
                            engine=inst.engine,
                            bass_nofuse=True,
                            sync_info=mybir.SyncInfo(on_wait=[w], on_update=[]),
                        )
                        new.append(nop)
                    si.on_wait = [waits[-1]]
                new.append(inst)
            bb.instructions[:] = new


def make_flip(nc, out, sq=P):
    # anti-identity: out[x, y] = 1 iff x + y == sq-1
    nc.gpsimd.memset(out, 0.0)
    nc.gpsimd.affine_select(
        out=out, in_=out,
        compare_op=mybir.AluOpType.not_equal,
        fill=1.0,
        base=-(sq - 1),
        pattern=[[1, sq]],
        channel_multiplier=1,
    )


def build_nc():
    nc = bass.Bass()
    x_d = nc.declare_dram_parameter("x", [C, N], FP32, isOutput=False)
    g_d = nc.declare_dram_parameter("gamma", [1, 1], FP32, isOutput=False)
    o_d = nc.declare_dram_parameter("out", [C, N], FP16, isOutput=True)

    # Clear kernel semaphores at START (idle window) instead of paying the
    # expensive teardown clear+barrier at the end (STRIP_TAIL above).
    # (Dropping this hangs the device — semaphore state persists across NEFF
    # loads.)
    from concourse.bass import compact_to_ranges

    for sem_range in compact_to_ranges(
        [sem for sem in nc._kernel_sem_range if sem not in nc.barrier_sems]
    ):
        nc.gpsimd.sem_clear(sem_range)
    nc._nrt_pseudo_barrier()

    with tile.TileContext(nc) as tc:
        with (
            tc.tile_pool(name="singles", bufs=1) as singles,
            tc.tile_pool(name="stage", bufs=4) as stage,
            tc.tile_pool(name="psum_acc", bufs=4, space="PSUM") as psum_acc,
            tc.tile_pool(name="psum_po", bufs=2, space="PSUM") as psum_po,
        ):
            # Constants on GPSIMD (free early); PE warm-up on a dep-free
            # tile; ACT Exp-table preload on a dummy.
            warm8 = singles.tile([P, P], FP8, tag="warm8")
            nc.gpsimd.memset(warm8[:], 1.0)
            J8 = singles.tile([P, P], FP8, tag="J8")
            make_flip(nc, J8[:])
            J32 = singles.tile([P, P], FP32, tag="J32")
            make_flip(nc, J32[:])
            gcol = singles.tile([P, 1], FP32, tag="gamma")
            nc.gpsimd.dma_start(out=gcol[:], in_=g_d[:, :].to_broadcast((P, 1)))

            for _ in range(12):
                wp = psum_po.tile([P, 1024], FP32, tag="po")
                nc.tensor.matmul(
                    wp[:, 0:P], lhsT=warm8[:], rhs=warm8[:], start=True, stop=True
                )
            dume = singles.tile([P, 1], FP32, tag="dume")
            nc.scalar.activation(
                out=dume[:], in_=warm8[:, 0:1], func=mybir.ActivationFunctionType.Exp
            )

            xf = [
                singles.tile([P, N], FP32, tag=f"xf{ci}", name=f"xf{ci}")
                for ci in range(CT)
            ]
            q8 = singles.tile([P, CT, N], FP8, tag="q8")
            qTp = singles.tile([P, CT, UT, P], FP16, tag="qTp")
            e_ps = [
                psum_acc.tile([P, C], FP32, tag="acc", name=f"e{ci}")
                for ci in range(CT)
            ]

            # Phase A per 1024-col group gg:
            #  - x chunk DMAs issued from the Vector queue (consumption order)
            #  - fp32->fp8 casts: ACT takes half 0, DVE takes half 1
            #  - XBAR transpose per (gg, ci): fp8 pairs viewed as fp16,
            #    [128, 512]f16 -> [128, 4, 128] into qTp[:, ci, gg*4:gg*4+4, :]
            #  - 16 energy DRI matmuls per gg (4 uchunks x 4 ci rows)
            for gg in range(GG):
                lo = gg * 1024
                for ci in range(CT):
                    nc.gpsimd.dma_start(
                        out=xf[ci][:, lo:lo + 1024],
                        in_=x_d[ci * P:(ci + 1) * P, lo:lo + 1024],
                    )
                for ci in range(CT):
                    nc.scalar.copy(
                        out=q8[:, ci, lo:lo + 512],
                        in_=xf[ci][:, lo:lo + 512],
                    )
                    nc.vector.tensor_copy(
                        out=q8[:, ci, lo + 512:lo + 1024],
                        in_=xf[ci][:, lo + 512:lo + 1024],
                    )
                for ci in range(CT):
                    nc.sync.dma_start_transpose(
                        out=qTp[:, ci, gg * 4:(gg + 1) * 4, :],
                        in_=q8[:, ci, lo:lo + 1024].bitcast(FP16),
                    )
                for tt in range(4):
                    t = gg * 4 + tt
                    rhs = qTp[:, :, t, :].bitcast(FP8).rearrange(
                        "p ci (c r) -> p r ci c", r=2
                    )
                    for ci in range(CT):
                        nc.tensor.matmul(
                            e_ps[ci][:],
                            lhsT=qTp[:, ci, t, :].bitcast(FP8),
                            rhs=rhs,
                            start=(t == 0),
                            stop=(t == UT - 1),
                            perf_mode=DRI,
                        )

            # Softmax per ci on the REVERSED rows: row min (DVE), exp with
            # fp8 out + row-sum accumulator (ACT), 1/Z and gamma/Z (DVE
            # small).  attT via fp8 matmuls against the FLIP matrix (undoes
            # the row reversal), ACT strided copy into EXPT.  ci=0 runs its
            # min/exp in halves to shorten the serial chain into att@q.
            mcol = singles.tile([P, CT], FP32, tag="m")
            mh = singles.tile([P, 2], FP32, tag="mh")
            zcol = singles.tile([P, CT], FP32, tag="z")
            zh = singles.tile([P, 2], FP32, tag="zh")
            lnz = singles.tile([P, CT], FP32, tag="lnz")
            bias2 = singles.tile([P, CT], FP32, tag="bias2")
            b2f = singles.tile([P, CT], FP32, tag="b2f")
            EXPQ = singles.tile([P, CT, C], FP8, tag="EXPQ")
            EXPT = singles.tile([P, CT, C], FP8, tag="EXPT")

            def softmax_head(ci):
                cs = slice(ci, ci + 1)
                if ci == 0:
                    nc.vector.tensor_reduce(
                        out=mh[:, 0:1], in_=e_ps[0][:, 0:256],
                        axis=mybir.AxisListType.X, op=MIN,
                    )
                    nc.vector.tensor_reduce(
                        out=mh[:, 1:2], in_=e_ps[0][:, 256:512],
                        axis=mybir.AxisListType.X, op=MIN,
                    )
                    nc.vector.tensor_tensor(
                        out=mcol[:, 0:1], in0=mh[:, 0:1], in1=mh[:, 1:2], op=MIN
                    )
                else:
                    nc.vector.tensor_reduce(
                        out=mcol[:, cs], in_=e_ps[ci][:],
                        axis=mybir.AxisListType.X, op=MIN,
                    )

            # b2ps: single [P, CT] PSUM tile (from the freed-energy-bank
            # ring) collecting the flipped bias2 columns as they appear.
            b2ps_holder = []

            def softmax_tail(ci):
                cs = slice(ci, ci + 1)
                if ci == 0:
                    for half in range(2):
                        nc.scalar.activation(
                            out=EXPQ[:, 0, half * 256:(half + 1) * 256],
                            in_=e_ps[0][:, half * 256:(half + 1) * 256],
                            func=mybir.ActivationFunctionType.Exp,
                            bias=mcol[:, 0:1],
                            scale=-1.0,
                            accum_out=zh[:, half:half + 1],
                        )
                    nc.vector.tensor_add(
                        out=zcol[:, 0:1], in0=zh[:, 0:1], in1=zh[:, 1:2]
                    )
                else:
                    nc.scalar.activation(
                        out=EXPQ[:, ci, :],
                        in_=e_ps[ci][:],
                        func=mybir.ActivationFunctionType.Exp,
                        bias=mcol[:, cs],
                        scale=-1.0,
                        accum_out=zcol[:, cs],
                    )
                nc.vector.reciprocal(out=lnz[:, cs], in_=zcol[:, cs])
                nc.vector.tensor_mul(out=bias2[:, cs], in0=lnz[:, cs], in1=gcol[:])
                # un-reverse bias2 rows: b2f[p] = bias2[127-p] (per column)
                if not b2ps_holder:
                    b2ps_holder.append(
                        psum_acc.tile([P, CT], FP32, tag="acc", name="b2ps")
                    )
                b2ps = b2ps_holder[0]
                nc.tensor.matmul(
                    b2ps[:, cs], lhsT=J32[:], rhs=bias2[:, cs],
                    start=True, stop=True,
                )
                nc.vector.tensor_copy(out=b2f[:, cs], in_=b2ps[:, cs])
                # attT transposes vs the flip matrix: EXPT columns come out
                # with FORWARD c.  All four land in quarters of one PSUM
                # bank, then one strided ACT copy scatters them into EXPT.
                ptx = psum_acc.tile([P, C], FP32, tag="acc", name=f"ptx{ci}")
                for dj in range(CT):
                    nc.tensor.matmul(
                        ptx[:, dj * P:(dj + 1) * P],
                        lhsT=EXPQ[:, ci, dj * P:(dj + 1) * P],
                        rhs=J8[:],
                        start=True,
                        stop=True,
                    )
                nc.scalar.copy(
                    out=EXPT[:, :, ci * P:(ci + 1) * P],
                    in_=ptx[:].rearrange("p (d j) -> p d j", d=CT),
                )

            # Drain engine per (ci, ng): True = DVE STT, False = ACT mul +
            # GPSIMD add.  10 DVE / 6 pair; the very last group is DVE.
            DRAIN_DVE = [
                [True, False, True, True],
                [False, True, False, True],
                [True, False, True, True],
                [False, True, True, True],
            ]

            def attq(ci):
                # att@q (DoubleRow, K=512 via dj pairs) into [P,1024] 2-bank
                # PSUM groups + fused epilogue out = po*(gamma/Z_c) + x,
                # fp16 store in two 2048-col chunks per ci.
                for nh in range(2):
                    osb = stage.tile([P, 2048], FP16, tag="osb")
                    for ng2 in range(2):
                        ng = nh * 2 + ng2
                        po = psum_po.tile([P, 1024], FP32, tag="po")
                        for sub in range(2):
                            nj = ng * 2 + sub
                            for j in range(2):
                                nc.tensor.matmul(
                                    po[:, sub * 512:(sub + 1) * 512],
                                    lhsT=EXPT[:, 2 * j:2 * j + 2, ci * P:(ci + 1) * P],
                                    rhs=q8[:, 2 * j:2 * j + 2, nj * 512:(nj + 1) * 512],
                                    start=(j == 0),
                                    stop=(j == 1),
                                    perf_mode=DR,
                                )
                        oslc = osb[:, ng2 * 1024:(ng2 + 1) * 1024]
                        xslc = xf[ci][:, ng * 1024:(ng + 1) * 1024]
                        if DRAIN_DVE[ci][ng]:
                            nc.vector.scalar_tensor_tensor(
                                out=oslc,
                                in0=po[:],
                                scalar=b2f[:, ci:ci + 1],
                                in1=xslc,
                                op0=mybir.AluOpType.mult,
                                op1=mybir.AluOpType.add,
                            )
                        else:
                            tmp = stage.tile([P, 1024], FP32, tag="tmp")
                            nc.scalar.mul(
                                out=tmp[:], in_=po[:], mul=b2f[:, ci:ci + 1]
                            )
                            nc.gpsimd.tensor_add(
                                out=oslc, in0=tmp[:], in1=xslc
                            )
                    nc.sync.dma_start(
                        out=o_d[ci * P:(ci + 1) * P, nh * 2048:(nh + 1) * 2048],
                        in_=osb[:],
                    )

            for ci in range(CT):
                softmax_head(ci)
            for ci in range(CT):
                softmax_tail(ci)
                attq(ci)

    _legalize_sync_waits(nc)
    return nc


def make_in_maps(x, gamma):
    x = np.ascontiguousarray(np.asarray(x, dtype=np.float32)).reshape(B, C, N)
    g = np.ascontiguousarray(np.asarray(gamma, dtype=np.float32)).reshape(1, 1)
    return [{"x": x[i], "gamma": g} for i in range(B)]


def kernel(x, y=None, gamma=None, **_ignored):
    from concourse.bass_utils import run_bass_kernel_spmd

    nc = build_nc()
    in_maps = make_in_maps(x, gamma)
    res = run_bass_kernel_spmd(nc, in_maps, list(range(B)))
    out = np.stack([np.asarray(res.results[i]["out"]) for i in range(B)])
    return out.reshape(B, C, 64, 64).astype(np.float32)


# revision 13
# speedup vs baseline: 1.6716x; 1.6716x over previous
"""CAM-module kernel for Trainium2, data-parallel over batch on 8 NeuronCores.

Per core (one batch sample, q = x[b] viewed as (C=512, N=4096) fp32):
  energy   = q @ q^T                      (C, C)   fp8 matmul, fp32 accum
  att[c,d] = exp(m_c - e[c,d]) / Z_c      with m_c = row min of energy
  out      = gamma * (att @ q) + x

The row-max shift of the reference softmax cancels algebraically; only the
row minimum is needed for numerical stability (arguments of exp stay <= 0).

Key structure (v2 — XBAR-transpose restructure):
 - q^T is produced by the DMA XBAR transpose engine (dma_start_transpose),
   viewing adjacent fp8 pairs as fp16.  This removes all 128 identity-matmul
   PE transposes and the 32 DVE PSUM->SBUF copies of the previous version.
   The XBAR output interleaves the two fp8 values of each pair along the
   free dim (A0 B0 A1 B1 ...), which is exactly the DoubleRowSwInterleave
   weight layout (A/B pairs per column, columns reversed), so the energy
   matmuls consume it directly:
     * lhsT = raw interleaved bytes of one 128-c block -> output rows come
       out REVERSED within each 128 block (c_local = 127 - p).
     * rhs  = byte-strided [p, r, (ci c)] view of the same tiles.
 - All row-wise softmax steps run unchanged on the reversed rows.  The
   reversal is undone for free in the attT transposes by using a FLIP
   (anti-identity) matrix instead of the identity as the moving operand;
   the per-row scale gamma/Z is un-reversed by one tiny fp32 matmul
   against a fp32 flip matrix (b2f = J @ bias2).
 - x loads are raw pre-tile DMAs with manual completion semaphores (the
   tile scheduler caps outstanding in-tile DMAs with a small sliding
   window, which would serialize loads behind XBAR completions).  Sync
   issues gg0-1 after clearing their sems itself; GPSIMD issues gg2-3
   after the general semaphore clear.  All 32 half-casts run on DVE
   (~426ns each), gated on the load sems post-scheduling; ACT issues the
   gg0 XBARs, sync the rest.
 - Phase C: att@q DoubleRow matmuls into [P,1024] 2-bank PSUM groups; the
   epilogue out = po*(gamma/Z) + x is drained by 1024-col ops balanced
   across DVE (scalar_tensor_tensor) and ACT-mul + GPSIMD-add pairs, fp16
   store in 8 x 1MB-ish DMAs.
 - PSUM: 4 banks energy accumulators (reused in phase C for the attT
   staging and the bias2 flip), 4 banks for the [P,1024] att@q groups.
"""

import numpy as np

import concourse.bass as bass
import concourse.tile as tile
from concourse import mybir
from concourse.vector_clock import ScopedClock

P = 128
C = 512
N = 4096
B = 8
CT = C // P   # 4 c-tiles
GG = 4        # 1024-col load groups
UT = N // 256  # 16 uchunks (256 n-values each)

STRIP_TAIL = True

FP32 = mybir.dt.float32
FP16 = mybir.dt.float16
FP8 = mybir.dt.float8e4
DR = mybir.MatmulPerfMode.DoubleRow
DRI = mybir.MatmulPerfMode.DoubleRowSwInterleave
MIN = mybir.AluOpType.min


def _drain_and_barrier_split(self, tick_clock, wait_clock):
    # The pinned walrus rejects >1 sync-wait on TPB_CTRL (Drain); spread the
    # final global-clock waits across a chain of drains, one wait each.
    nc = self.nc
    drain_inst = nc.sync.drain()
    wait_clock.add_sem_waits(
        drain_inst.ins, ScopedClock({None: tick_clock.global_clock})
    )
    si = drain_inst.ins.sync_info
    if si is not None and si.on_wait is not None and len(si.on_wait) > 1:
        waits = list(si.on_wait)
        si.on_wait = waits[:1]
        for w in waits[1:]:
            extra = nc.sync.drain()
            extra.ins.sync_info = mybir.SyncInfo(on_wait=[w], on_update=[])
    nc.all_engine_barrier()
    assert self.sems is not None
    popped = nc._tile_sem_poison_stack.pop()
    assert popped is self._sem_poison
    if not STRIP_TAIL:
        nc.clear_and_free_semaphores(list(self.sems.allocated().values()))
        nc.all_engine_barrier()


tile.TileContext._drain_and_barrier = _drain_and_barrier_split


def _legalize_sync_waits(nc):
    # This walrus build rejects instructions carrying more than one sync-wait.
    # Hoist extra waits onto same-engine NoOps placed immediately before the
    # instruction (engine streams preserve relative order within a block).
    for f in nc.m.functions:
        for bb in f.blocks:
            new = []
            for inst in bb.instructions:
                si = inst.sync_info
                if si is not None and si.on_wait and len(si.on_wait) > 1:
                    waits = list(si.on_wait)
                    for w in waits[:-1]:
                        nop = mybir.InstNoOp(
                            name=nc.get_next_instruction_name(),
                            engine=inst.engine,
                            bass_nofuse=True,
                            sync_info=mybir.SyncInfo(on_wait=[w], on_update=[]),
                        )
                        new.append(nop)
                    si.on_wait = [waits[-1]]
                new.append(inst)
            bb.instructions[:] = new


def make_flip(nc, out, sq=P):
    # anti-identity: out[x, y] = 1 iff x + y == sq-1
    nc.gpsimd.memset(out, 0.0)
    nc.gpsimd.affine_select(
        out=out, in_=out,
        compare_op=mybir.AluOpType.not_equal,
        fill=1.0,
        base=-(sq - 1),
        pattern=[[1, sq]],
        channel_multiplier=1,
    )


def build_nc():
    nc = bass.Bass()
    x_d = nc.declare_dram_parameter("x", [C, N], FP32, isOutput=False)
    g_d = nc.declare_dram_parameter("gamma", [1, 1], FP32, isOutput=False)
    o_d = nc.declare_dram_parameter("out", [C, N], FP16, isOutput=True)

    # x-load completion semaphores, one per (gg, ci) chunk.
    xsem = [[nc.alloc_semaphore(f"xld{gg}_{ci}") for ci in range(CT)] for gg in range(GG)]
    xf = [nc.alloc_sbuf_tensor(f"xraw{ci}", [P, N], FP32) for ci in range(CT)]

    # Clear kernel semaphores at START (idle window) instead of paying the
    # expensive teardown clear+barrier at the end (STRIP_TAIL above).
    # (Dropping this hangs the device — semaphore state persists across NEFF
    # loads.)  Sync clears the gg0-1 load sems itself (no barrier wait) and
    # starts those loads immediately; GPSIMD's general clear covers the rest
    # (including the gg2-3 load sems, which GPSIMD itself issues afterwards,
    # so same-queue ordering protects them).
    from concourse.bass import compact_to_ranges

    early_sems = {xsem[gg][ci].num for gg in range(2) for ci in range(CT)}
    for sem_range in compact_to_ranges(sorted(early_sems)):
        nc.sync.sem_clear(sem_range)
    for gg in range(2):
        for ci in range(CT):
            nc.sync.dma_start(
                out=xf[ci][:, gg * 1024:(gg + 1) * 1024],
                in_=x_d[ci * P:(ci + 1) * P, gg * 1024:(gg + 1) * 1024],
            ).then_inc(xsem[gg][ci], 16)

    for sem_range in compact_to_ranges(
        [
            sem
            for sem in nc._kernel_sem_range
            if sem not in nc.barrier_sems and int(sem) not in early_sems
        ]
    ):
        nc.gpsimd.sem_clear(sem_range)
    nc._nrt_pseudo_barrier()
    for gg in range(2, GG):
        for ci in range(CT):
            nc.gpsimd.dma_start(
                out=xf[ci][:, gg * 1024:(gg + 1) * 1024],
                in_=x_d[ci * P:(ci + 1) * P, gg * 1024:(gg + 1) * 1024],
            ).then_inc(xsem[gg][ci], 16)

    cast_waits = []  # (BassInstruction, gg, ci): xsem waits attached post-scheduling

    with tile.TileContext(nc) as tc:
        with (
            tc.tile_pool(name="singles", bufs=1) as singles,
            tc.tile_pool(name="stage", bufs=4) as stage,
            tc.tile_pool(name="psum_acc", bufs=4, space="PSUM") as psum_acc,
            tc.tile_pool(name="psum_po", bufs=2, space="PSUM") as psum_po,
        ):
            # Constants on GPSIMD (free early); PE warm-up on a dep-free
            # tile; ACT Exp-table preload on a dummy.
            warm8 = singles.tile([P, P], FP8, tag="warm8")
            nc.vector.memset(warm8[:], 1.0)
            J8 = singles.tile([P, P], FP8, tag="J8")
            make_flip(nc, J8[:])
            J32 = singles.tile([P, P], FP32, tag="J32")
            make_flip(nc, J32[:])
            gcol = singles.tile([P, 1], FP32, tag="gamma")
            nc.gpsimd.dma_start(out=gcol[:], in_=g_d[:, :].to_broadcast((P, 1)))

            for _ in range(12):
                wp = psum_po.tile([P, 1024], FP32, tag="po")
                nc.tensor.matmul(
                    wp[:, 0:P], lhsT=warm8[:], rhs=warm8[:], start=True, stop=True
                )
            dume = singles.tile([P, 1], FP32, tag="dume")
            nc.scalar.activation(
                out=dume[:], in_=warm8[:, 0:1], func=mybir.ActivationFunctionType.Exp
            )

            q8 = singles.tile([P, CT, N], FP8, tag="q8")
            qTp = singles.tile([P, CT, UT, P], FP16, tag="qTp")
            e_ps = [
                psum_acc.tile([P, C], FP32, tag="acc", name=f"e{ci}")
                for ci in range(CT)
            ]

            # Phase A per 1024-col group gg:
            #  - fp32->fp8 casts both halves on DVE, gated on the raw load
            #    sems (attached after tile scheduling)
            #  - XBAR transpose per (gg, ci): fp8 pairs viewed as fp16,
            #    [128, 512]f16 -> [128, 4, 128] into qTp[:, ci, gg*4:gg*4+4, :]
            #    (ACT issues gg0 — sync is still busy issuing the loads)
            #  - 16 energy DRI matmuls per gg (4 uchunks x 4 ci rows)
            for gg in range(GG):
                lo = gg * 1024
                for ci in range(CT):
                    for half in range(2):
                        cst = nc.vector.tensor_copy(
                            out=q8[:, ci, lo + half * 512:lo + (half + 1) * 512],
                            in_=xf[ci][:, lo + half * 512:lo + (half + 1) * 512],
                        )
                        cast_waits.append((cst, gg, ci))
                xbar_eng = nc.scalar if gg == 0 else nc.sync
                for ci in range(CT):
                    xbar_eng.dma_start_transpose(
                        out=qTp[:, ci, gg * 4:(gg + 1) * 4, :],
                        in_=q8[:, ci, lo:lo + 1024].bitcast(FP16),
                    )
                for tt in range(4):
                    t = gg * 4 + tt
                    rhs = qTp[:, :, t, :].bitcast(FP8).rearrange(
                        "p ci (c r) -> p r ci c", r=2
                    )
                    for ci in range(CT):
                        nc.tensor.matmul(
                            e_ps[ci][:],
                            lhsT=qTp[:, ci, t, :].bitcast(FP8),
                            rhs=rhs,
                            start=(t == 0),
                            stop=(t == UT - 1),
                            perf_mode=DRI,
                        )

            # Softmax per ci on the REVERSED rows: row min (DVE), exp with
            # fp8 out + row-sum accumulator (ACT), 1/Z and gamma/Z (DVE
            # small).  attT via fp8 matmuls against the FLIP matrix (undoes
            # the row reversal), ACT strided copy into EXPT.  ci=0 runs its
            # min/exp in halves to shorten the serial chain into att@q.
            mcol = singles.tile([P, CT], FP32, tag="m")
            mh = singles.tile([P, 2], FP32, tag="mh")
            zcol = singles.tile([P, CT], FP32, tag="z")
            zh = singles.tile([P, 2], FP32, tag="zh")
            lnz = singles.tile([P, CT], FP32, tag="lnz")
            bias2 = singles.tile([P, CT], FP32, tag="bias2")
            b2f = singles.tile([P, CT], FP32, tag="b2f")
            EXPQ = singles.tile([P, CT, C], FP8, tag="EXPQ")
            EXPT = singles.tile([P, CT, C], FP8, tag="EXPT")

            def softmax_head(ci):
                cs = slice(ci, ci + 1)
                if ci == 0:
                    nc.vector.tensor_reduce(
                        out=mh[:, 0:1], in_=e_ps[0][:, 0:256],
                        axis=mybir.AxisListType.X, op=MIN,
                    )
                    nc.vector.tensor_reduce(
                        out=mh[:, 1:2], in_=e_ps[0][:, 256:512],
                        axis=mybir.AxisListType.X, op=MIN,
                    )
                    nc.vector.tensor_tensor(
                        out=mcol[:, 0:1], in0=mh[:, 0:1], in1=mh[:, 1:2], op=MIN
                    )
                else:
                    nc.vector.tensor_reduce(
                        out=mcol[:, cs], in_=e_ps[ci][:],
                        axis=mybir.AxisListType.X, op=MIN,
                    )

            # b2ps: single [P, CT] PSUM tile (from the freed-energy-bank
            # ring) collecting the flipped bias2 columns as they appear.
            b2ps_holder = []

            def softmax_tail(ci):
                cs = slice(ci, ci + 1)
                if ci == 0:
                    for half in range(2):
                        nc.scalar.activation(
                            out=EXPQ[:, 0, half * 256:(half + 1) * 256],
                            in_=e_ps[0][:, half * 256:(half + 1) * 256],
                            func=mybir.ActivationFunctionType.Exp,
                            bias=mcol[:, 0:1],
                            scale=-1.0,
                            accum_out=zh[:, half:half + 1],
                        )
                    nc.vector.tensor_add(
                        out=zcol[:, 0:1], in0=zh[:, 0:1], in1=zh[:, 1:2]
                    )
                else:
                    nc.scalar.activation(
                        out=EXPQ[:, ci, :],
                        in_=e_ps[ci][:],
                        func=mybir.ActivationFunctionType.Exp,
                        bias=mcol[:, cs],
                        scale=-1.0,
                        accum_out=zcol[:, cs],
                    )
                nc.vector.reciprocal(out=lnz[:, cs], in_=zcol[:, cs])
                nc.vector.tensor_mul(out=bias2[:, cs], in0=lnz[:, cs], in1=gcol[:])
                # un-reverse bias2 rows: b2f[p] = bias2[127-p] (per column)
                if not b2ps_holder:
                    b2ps_holder.append(
                        psum_acc.tile([P, CT], FP32, tag="acc", name="b2ps")
                    )
                b2ps = b2ps_holder[0]
                nc.tensor.matmul(
                    b2ps[:, cs], lhsT=J32[:], rhs=bias2[:, cs],
                    start=True, stop=True,
                )
                nc.vector.tensor_copy(out=b2f[:, cs], in_=b2ps[:, cs])
                # attT transposes vs the flip matrix: EXPT columns come out
                # with FORWARD c.  All four land in quarters of one PSUM
                # bank, then one strided ACT copy scatters them into EXPT.
                ptx = psum_acc.tile([P, C], FP32, tag="acc", name=f"ptx{ci}")
                for dj in range(CT):
                    nc.tensor.matmul(
                        ptx[:, dj * P:(dj + 1) * P],
                        lhsT=EXPQ[:, ci, dj * P:(dj + 1) * P],
                        rhs=J8[:],
                        start=True,
                        stop=True,
                    )
                nc.scalar.copy(
                    out=EXPT[:, :, ci * P:(ci + 1) * P],
                    in_=ptx[:].rearrange("p (d j) -> p d j", d=CT),
                )

            # Drain engine per (ci, ng): True = DVE STT, False = ACT mul +
            # GPSIMD add.  10 DVE / 6 pair; the very last group is DVE.
            DRAIN_DVE = [
                [True, False, True, True],
                [False, True, False, True],
                [True, False, True, True],
                [False, True, True, True],
            ]

            def attq(ci):
                # att@q (DoubleRow, K=512 via dj pairs) into [P,1024] 2-bank
                # PSUM groups + fused epilogue out = po*(gamma/Z_c) + x,
                # fp16 store in two 2048-col chunks per ci.
                for nh in range(2):
                    osb = stage.tile([P, 2048], FP16, tag="osb")
                    for ng2 in range(2):
                        ng = nh * 2 + ng2
                        po = psum_po.tile([P, 1024], FP32, tag="po")
                        for sub in range(2):
                            nj = ng * 2 + sub
                            for j in range(2):
                                nc.tensor.matmul(
                                    po[:, sub * 512:(sub + 1) * 512],
                                    lhsT=EXPT[:, 2 * j:2 * j + 2, ci * P:(ci + 1) * P],
                                    rhs=q8[:, 2 * j:2 * j + 2, nj * 512:(nj + 1) * 512],
                                    start=(j == 0),
                                    stop=(j == 1),
                                    perf_mode=DR,
                                )
                        oslc = osb[:, ng2 * 1024:(ng2 + 1) * 1024]
                        xslc = xf[ci][:, ng * 1024:(ng + 1) * 1024]
                        if DRAIN_DVE[ci][ng]:
                            nc.vector.scalar_tensor_tensor(
                                out=oslc,
                                in0=po[:],
                                scalar=b2f[:, ci:ci + 1],
                                in1=xslc,
                                op0=mybir.AluOpType.mult,
                                op1=mybir.AluOpType.add,
                            )
                        else:
                            tmp = stage.tile([P, 1024], FP32, tag="tmp")
                            nc.scalar.mul(
                                out=tmp[:], in_=po[:], mul=b2f[:, ci:ci + 1]
                            )
                            nc.gpsimd.tensor_add(
                                out=oslc, in0=tmp[:], in1=xslc
                            )
                    nc.sync.dma_start(
                        out=o_d[ci * P:(ci + 1) * P, nh * 2048:(nh + 1) * 2048],
                        in_=osb[:],
                    )

            for ci in range(CT):
                softmax_head(ci)
            for ci in range(CT):
                softmax_tail(ci)
                attq(ci)

    # The raw-load gating is invisible to the tile scheduler (its deadlock
    # simulator would stall on semaphores no in-context instruction bumps),
    # so attach the waits only after scheduling has run.
    for cst, gg, ci in cast_waits:
        cst.wait_op(xsem[gg][ci], 16, "sem-ge")
    _legalize_sync_waits(nc)
    return nc


def make_in_maps(x, gamma):
    x = np.ascontiguousarray(np.asarray(x, dtype=np.float32)).reshape(B, C, N)
    g = np.ascontiguousarray(np.asarray(gamma, dtype=np.float32)).reshape(1, 1)
    return [{"x": x[i], "gamma": g} for i in range(B)]


def kernel(x, y=None, gamma=None, **_ignored):
    from concourse.bass_utils import run_bass_kernel_spmd

    nc = build_nc()
    in_maps = make_in_maps(x, gamma)
    res = run_bass_kernel_spmd(nc, in_maps, list(range(B)))
    out = np.stack([np.asarray(res.results[i]["out"]) for i in range(B)])
    return out.reshape(B, C, 64, 64).astype(np.float32)


# revision 21
# speedup vs baseline: 1.7351x; 1.0380x over previous
"""CAM-module kernel for Trainium2, data-parallel over batch on 8 NeuronCores.

Per core (one batch sample, q = x[b] viewed as (C=512, N=4096) fp32):
  energy   = q @ q^T                      (C, C)   fp8 matmul, fp32 accum
  att[c,d] = exp(m_c - e[c,d]) / Z_c      with m_c = row min of energy
  out      = gamma * (att @ q) + x

The row-max shift of the reference softmax cancels algebraically; only the
row minimum is needed for numerical stability (arguments of exp stay <= 0).

Key structure (v2 — XBAR-transpose restructure):
 - q^T is produced by the DMA XBAR transpose engine (dma_start_transpose),
   viewing adjacent fp8 pairs as fp16.  This removes all 128 identity-matmul
   PE transposes and the 32 DVE PSUM->SBUF copies of the previous version.
   The XBAR output interleaves the two fp8 values of each pair along the
   free dim (A0 B0 A1 B1 ...), which is exactly the DoubleRowSwInterleave
   weight layout (A/B pairs per column, columns reversed), so the energy
   matmuls consume it directly:
     * lhsT = raw interleaved bytes of one 128-c block -> output rows come
       out REVERSED within each 128 block (c_local = 127 - p).
     * rhs  = byte-strided [p, r, (ci c)] view of the same tiles.
 - All row-wise softmax steps run unchanged on the reversed rows.  The
   reversal is undone for free in the attT transposes by using a FLIP
   (anti-identity) matrix instead of the identity as the moving operand;
   the per-row scale gamma/Z is un-reversed by one tiny fp32 matmul
   against a fp32 flip matrix (b2f = J @ bias2).
 - x loads are raw pre-tile DMAs with manual completion semaphores (the
   tile scheduler caps outstanding in-tile DMAs with a small sliding
   window, which would serialize loads behind XBAR completions).  Sync
   issues gg0-1 after clearing their sems itself; GPSIMD issues gg2-3
   after the general semaphore clear.  All 32 half-casts run on DVE
   (~426ns each), gated on the load sems post-scheduling; ACT issues the
   gg0 XBARs, sync the rest.
 - Phase C: att@q DoubleRow matmuls into [P,1024] 2-bank PSUM groups; the
   epilogue out = po*(gamma/Z) + x is drained by 1024-col ops balanced
   across DVE (scalar_tensor_tensor) and ACT-mul + GPSIMD-add pairs, fp16
   store in 8 x 1MB-ish DMAs.
 - PSUM: 4 banks energy accumulators (reused in phase C for the attT
   staging and the bias2 flip), 4 banks for the [P,1024] att@q groups.
"""

import numpy as np

import concourse.bass as bass
import concourse.tile as tile
from concourse import mybir
from concourse.vector_clock import ScopedClock

P = 128
C = 512
N = 4096
B = 8
CT = C // P   # 4 c-tiles
GG = 4        # 1024-col load groups
UT = N // 256  # 16 uchunks (256 n-values each)

STRIP_TAIL = True

FP32 = mybir.dt.float32
FP16 = mybir.dt.float16
FP8 = mybir.dt.float8e4
DR = mybir.MatmulPerfMode.DoubleRow
DRI = mybir.MatmulPerfMode.DoubleRowSwInterleave
MIN = mybir.AluOpType.min


def _drain_and_barrier_split(self, tick_clock, wait_clock):
    # The pinned walrus rejects >1 sync-wait on TPB_CTRL (Drain); spread the
    # final global-clock waits across a chain of drains, one wait each.
    nc = self.nc
    drain_inst = nc.sync.drain()
    wait_clock.add_sem_waits(
        drain_inst.ins, ScopedClock({None: tick_clock.global_clock})
    )
    si = drain_inst.ins.sync_info
    if si is not None and si.on_wait is not None and len(si.on_wait) > 1:
        waits = list(si.on_wait)
        si.on_wait = waits[:1]
        for w in waits[1:]:
            extra = nc.sync.drain()
            extra.ins.sync_info = mybir.SyncInfo(on_wait=[w], on_update=[])
    nc.all_engine_barrier()
    assert self.sems is not None
    popped = nc._tile_sem_poison_stack.pop()
    assert popped is self._sem_poison
    if not STRIP_TAIL:
        nc.clear_and_free_semaphores(list(self.sems.allocated().values()))
        nc.all_engine_barrier()


tile.TileContext._drain_and_barrier = _drain_and_barrier_split


def _legalize_sync_waits(nc):
    # This walrus build rejects instructions carrying more than one sync-wait.
    # Hoist extra waits onto same-engine NoOps placed immediately before the
    # instruction (engine streams preserve relative order within a block).
    for f in nc.m.functions:
        for bb in f.blocks:
            new = []
            for inst in bb.instructions:
                si = inst.sync_info
                if si is not None and si.on_wait and len(si.on_wait) > 1:
                    waits = list(si.on_wait)
                    for w in waits[:-1]:
                        nop = mybir.InstNoOp(
                            name=nc.get_next_instruction_name(),
                            engine=inst.engine,
                            bass_nofuse=True,
                            sync_info=mybir.SyncInfo(on_wait=[w], on_update=[]),
                        )
                        new.append(nop)
                    si.on_wait = [waits[-1]]
                new.append(inst)
            bb.instructions[:] = new


def make_flip(nc, out, sq=P):
    # anti-identity: out[x, y] = 1 iff x + y == sq-1
    nc.gpsimd.memset(out, 0.0)
    nc.gpsimd.affine_select(
        out=out, in_=out,
        compare_op=mybir.AluOpType.not_equal,
        fill=1.0,
        base=-(sq - 1),
        pattern=[[1, sq]],
        channel_multiplier=1,
    )


def build_nc():
    nc = bass.Bass()
    x_d = nc.declare_dram_parameter("x", [C, N], FP32, isOutput=False)
    g_d = nc.declare_dram_parameter("gamma", [1, 1], FP32, isOutput=False)
    o_d = nc.declare_dram_parameter("out", [C, N], FP16, isOutput=True)

    # x-load completion semaphores, one per (gg, ci) chunk, plus one for the
    # gamma broadcast.
    xsem = [[nc.alloc_semaphore(f"xld{gg}_{ci}") for ci in range(CT)] for gg in range(GG)]
    gsem = nc.alloc_semaphore("gld")
    xf = [nc.alloc_sbuf_tensor(f"xraw{ci}", [P, N], FP32) for ci in range(CT)]
    gcol_raw = nc.alloc_sbuf_tensor("gcolraw", [P, 1], FP32)

    # Clear kernel semaphores at START (idle window) instead of paying the
    # expensive teardown clear+barrier at the end (STRIP_TAIL above).
    # (Dropping this hangs the device — semaphore state persists across NEFF
    # loads.)  All raw loads are issued AFTER the pseudo barrier so no engine
    # waits on another's issue backlog: sync takes gg0-1, gpsimd gg2-3 +
    # gamma.  In-tile consumers get explicit sem waits after scheduling.
    from concourse.bass import compact_to_ranges

    for sem_range in compact_to_ranges(
        [sem for sem in nc._kernel_sem_range if sem not in nc.barrier_sems]
    ):
        nc.gpsimd.sem_clear(sem_range)
    nc._nrt_pseudo_barrier()
    for gg in range(2):
        for ci in range(CT):
            nc.sync.dma_start(
                out=xf[ci][:, gg * 1024:(gg + 1) * 1024],
                in_=x_d[ci * P:(ci + 1) * P, gg * 1024:(gg + 1) * 1024],
            ).then_inc(xsem[gg][ci], 16)
    nc.gpsimd.dma_start(
        out=gcol_raw[:, :], in_=g_d[:, :].to_broadcast((P, 1))
    ).then_inc(gsem, 16)
    for gg in range(2, GG):
        for ci in range(CT):
            nc.gpsimd.dma_start(
                out=xf[ci][:, gg * 1024:(gg + 1) * 1024],
                in_=x_d[ci * P:(ci + 1) * P, gg * 1024:(gg + 1) * 1024],
            ).then_inc(xsem[gg][ci], 16)

    cast_waits = []  # (BassInstruction, gg, ci): xsem waits attached post-scheduling
    gcol_waits = []  # instructions reading gcol_raw: gsem waits attached post-scheduling

    with tile.TileContext(nc) as tc:
        with (
            tc.tile_pool(name="singles", bufs=1) as singles,
            tc.tile_pool(name="stage", bufs=4) as stage,
            tc.tile_pool(name="psum_acc", bufs=4, space="PSUM") as psum_acc,
            tc.tile_pool(name="psum_po", bufs=3, space="PSUM") as psum_po,
            tc.tile_pool(name="psum_flip", bufs=1, space="PSUM") as psum_flip,
        ):
            # Constants on GPSIMD (free early); PE warm-up on a dep-free
            # tile; ACT Exp-table preload on a dummy.
            warm8 = singles.tile([P, P], FP8, tag="warm8")
            nc.vector.memset(warm8[:], 1.0)
            J8 = singles.tile([P, P], FP8, tag="J8")
            make_flip(nc, J8[:])
            J32 = singles.tile([P, P], FP32, tag="J32")
            make_flip(nc, J32[:])
            gcol = singles.tile([P, 1], FP32, tag="gamma")
            gcp = nc.vector.tensor_copy(out=gcol[:], in_=gcol_raw[:, :])
            gcol_waits.append(gcp)

            for _ in range(12):
                wp = psum_po.tile([P, 512], FP32, tag="po")
                nc.tensor.matmul(
                    wp[:, 0:P], lhsT=warm8[:], rhs=warm8[:], start=True, stop=True
                )
            dume = singles.tile([P, 1], FP32, tag="dume")
            nc.scalar.activation(
                out=dume[:], in_=warm8[:, 0:1], func=mybir.ActivationFunctionType.Exp
            )

            q8 = singles.tile([P, CT, N], FP8, tag="q8")
            qTp = singles.tile([P, CT, UT, P], FP16, tag="qTp")
            e_ps = [
                psum_acc.tile([P, C], FP32, tag="acc", name=f"e{ci}")
                for ci in range(CT)
            ]

            # Phase A per 1024-col group gg:
            #  - fp32->fp8 casts both halves on DVE, gated on the raw load
            #    sems (attached after tile scheduling)
            #  - XBAR transpose per (gg, ci): fp8 pairs viewed as fp16,
            #    [128, 512]f16 -> [128, 4, 128] into qTp[:, ci, gg*4:gg*4+4, :]
            #    (ACT issues gg0 — sync is still busy issuing the loads)
            #  - 16 energy DRI matmuls per gg (4 uchunks x 4 ci rows)
            for gg in range(GG):
                lo = gg * 1024
                for ci in range(CT):
                    for half in range(2):
                        cst = nc.vector.tensor_copy(
                            out=q8[:, ci, lo + half * 512:lo + (half + 1) * 512],
                            in_=xf[ci][:, lo + half * 512:lo + (half + 1) * 512],
                        )
                        cast_waits.append((cst, gg, ci))
                xbar_eng = nc.scalar if gg == 0 else nc.sync
                for ci in range(CT):
                    xbar_eng.dma_start_transpose(
                        out=qTp[:, ci, gg * 4:(gg + 1) * 4, :],
                        in_=q8[:, ci, lo:lo + 1024].bitcast(FP16),
                    )
                for tt in range(4):
                    t = gg * 4 + tt
                    rhs = qTp[:, :, t, :].bitcast(FP8).rearrange(
                        "p ci (c r) -> p r ci c", r=2
                    )
                    for ci in range(CT):
                        nc.tensor.matmul(
                            e_ps[ci][:],
                            lhsT=qTp[:, ci, t, :].bitcast(FP8),
                            rhs=rhs,
                            start=(t == 0),
                            stop=(t == UT - 1),
                            perf_mode=DRI,
                        )

            # Softmax per ci on the REVERSED rows: row min (DVE), exp with
            # fp8 out + row-sum accumulator (ACT), 1/Z and gamma/Z (DVE
            # small).  attT via fp8 matmuls against the FLIP matrix (undoes
            # the row reversal), ACT strided copy into EXPT.  ci=0 runs its
            # min/exp in halves to shorten the serial chain into att@q.
            mcol = singles.tile([P, CT], FP32, tag="m")
            mh = singles.tile([P, 2], FP32, tag="mh")
            zcol = singles.tile([P, CT], FP32, tag="z")
            zh = singles.tile([P, 2], FP32, tag="zh")
            lnz = singles.tile([P, CT], FP32, tag="lnz")
            bias2 = singles.tile([P, CT], FP32, tag="bias2")
            b2f = singles.tile([P, CT], FP32, tag="b2f")
            EXPQ = singles.tile([P, CT, C], FP8, tag="EXPQ")
            EXPT = singles.tile([P, CT, C], FP8, tag="EXPT")

            def softmax_head(ci):
                cs = slice(ci, ci + 1)
                if ci == 0:
                    nc.vector.tensor_reduce(
                        out=mh[:, 0:1], in_=e_ps[0][:, 0:256],
                        axis=mybir.AxisListType.X, op=MIN,
                    )
                    nc.vector.tensor_reduce(
                        out=mh[:, 1:2], in_=e_ps[0][:, 256:512],
                        axis=mybir.AxisListType.X, op=MIN,
                    )
                    nc.vector.tensor_tensor(
                        out=mcol[:, 0:1], in0=mh[:, 0:1], in1=mh[:, 1:2], op=MIN
                    )
                else:
                    nc.vector.tensor_reduce(
                        out=mcol[:, cs], in_=e_ps[ci][:],
                        axis=mybir.AxisListType.X, op=MIN,
                    )

            # b2ps: single [P, CT] PSUM tile (from the freed-energy-bank
            # ring) collecting the flipped bias2 columns as they appear.
            b2ps_holder = []

            def softmax_tail(ci):
                cs = slice(ci, ci + 1)
                if ci == 0:
                    for half in range(2):
                        nc.scalar.activation(
                            out=EXPQ[:, 0, half * 256:(half + 1) * 256],
                            in_=e_ps[0][:, half * 256:(half + 1) * 256],
                            func=mybir.ActivationFunctionType.Exp,
                            bias=mcol[:, 0:1],
                            scale=-1.0,
                            accum_out=zh[:, half:half + 1],
                        )
                    nc.vector.tensor_add(
                        out=zcol[:, 0:1], in0=zh[:, 0:1], in1=zh[:, 1:2]
                    )
                else:
                    nc.scalar.activation(
                        out=EXPQ[:, ci, :],
                        in_=e_ps[ci][:],
                        func=mybir.ActivationFunctionType.Exp,
                        bias=mcol[:, cs],
                        scale=-1.0,
                        accum_out=zcol[:, cs],
                    )
                nc.vector.reciprocal(out=lnz[:, cs], in_=zcol[:, cs])
                nc.vector.tensor_mul(out=bias2[:, cs], in0=lnz[:, cs], in1=gcol[:])
                # un-reverse bias2 rows: b2f[p] = bias2[127-p] (per column)
                if not b2ps_holder:
                    b2ps_holder.append(
                        psum_flip.tile([P, CT], FP32, tag="b2ps", name="b2ps")
                    )
                b2ps = b2ps_holder[0]
                nc.tensor.matmul(
                    b2ps[:, cs], lhsT=J32[:], rhs=bias2[:, cs],
                    start=True, stop=True,
                )
                nc.vector.tensor_copy(out=b2f[:, cs], in_=b2ps[:, cs])
                # attT transposes vs the flip matrix: EXPT columns come out
                # with FORWARD c.  All four land in quarters of one PSUM
                # bank, then one strided ACT copy scatters them into EXPT.
                ptx = psum_acc.tile([P, C], FP32, tag="acc", name=f"ptx{ci}")
                for dj in range(CT):
                    nc.tensor.matmul(
                        ptx[:, dj * P:(dj + 1) * P],
                        lhsT=EXPQ[:, ci, dj * P:(dj + 1) * P],
                        rhs=J8[:],
                        start=True,
                        stop=True,
                    )
                nc.scalar.copy(
                    out=EXPT[:, :, ci * P:(ci + 1) * P],
                    in_=ptx[:].rearrange("p (d j) -> p d j", d=CT),
                )

            def attq(ci):
                # att@q (DoubleRow, K=512 via dj pairs) + fused epilogue add
                # out = po * (gamma/Z_c) + x, fp16 store.  3 subs drain via
                # DVE scalar_tensor_tensor, the 4th via ACT-scale +
                # GPSIMD-add; sub 3 reuses a freed energy-accumulator bank
                # so the matmuls never stall on the adds.
                for nh in range(2):
                    osb = stage.tile([P, 2048], FP16, tag="osb")
                    for sub in range(4):
                        nj = nh * 4 + sub
                        if sub == 3:
                            po = psum_acc.tile([P, 512], FP32, tag="acc", name="po")
                        else:
                            po = psum_po.tile([P, 512], FP32, tag="po")
                        for j in range(2):
                            nc.tensor.matmul(
                                po[:],
                                lhsT=EXPT[:, 2 * j:2 * j + 2, ci * P:(ci + 1) * P],
                                rhs=q8[:, 2 * j:2 * j + 2, nj * 512:(nj + 1) * 512],
                                start=(j == 0),
                                stop=(j == 1),
                                perf_mode=DR,
                            )
                        if sub < 3:
                            nc.vector.scalar_tensor_tensor(
                                out=osb[:, sub * 512:(sub + 1) * 512],
                                in0=po[:],
                                scalar=b2f[:, ci:ci + 1],
                                in1=xf[ci][:, nj * 512:(nj + 1) * 512],
                                op0=mybir.AluOpType.mult,
                                op1=mybir.AluOpType.add,
                            )
                        else:
                            tmp = stage.tile([P, 512], FP32, tag="tmp")
                            nc.scalar.mul(
                                out=tmp[:], in_=po[:], mul=b2f[:, ci:ci + 1]
                            )
                            nc.gpsimd.tensor_add(
                                out=osb[:, sub * 512:(sub + 1) * 512],
                                in0=tmp[:],
                                in1=xf[ci][:, nj * 512:(nj + 1) * 512],
                            )
                    nc.sync.dma_start(
                        out=o_d[ci * P:(ci + 1) * P, nh * 2048:(nh + 1) * 2048],
                        in_=osb[:],
                    )

            for ci in range(CT):
                softmax_head(ci)
            for ci in range(CT):
                softmax_tail(ci)
                attq(ci)

    # The raw-load gating is invisible to the tile scheduler (its deadlock
    # simulator would stall on semaphores no in-context instruction bumps),
    # so attach the waits only after scheduling has run.
    for cst, gg, ci in cast_waits:
        cst.wait_op(xsem[gg][ci], 16, "sem-ge")
    for ins in gcol_waits:
        ins.wait_op(gsem, 16, "sem-ge")
    _legalize_sync_waits(nc)
    return nc


def make_in_maps(x, gamma):
    x = np.ascontiguousarray(np.asarray(x, dtype=np.float32)).reshape(B, C, N)
    g = np.ascontiguousarray(np.asarray(gamma, dtype=np.float32)).reshape(1, 1)
    return [{"x": x[i], "gamma": g} for i in range(B)]


def kernel(x, y=None, gamma=None, **_ignored):
    from concourse.bass_utils import run_bass_kernel_spmd

    nc = build_nc()
    in_maps = make_in_maps(x, gamma)
    res = run_bass_kernel_spmd(nc, in_maps, list(range(B)))
    out = np.stack([np.asarray(res.results[i]["out"]) for i in range(B)])
    return out.reshape(B, C, 64, 64).astype(np.float32)


# revision 25
# speedup vs baseline: 1.7627x; 1.0159x over previous
"""CAM-module kernel for Trainium2, data-parallel over batch on 8 NeuronCores.

Per core (one batch sample, q = x[b] viewed as (C=512, N=4096) fp32):
  energy   = q @ q^T                      (C, C)   fp8 matmul, fp32 accum
  att[c,d] = exp(m_c - e[c,d]) / Z_c      with m_c = row min of energy
  out      = gamma * (att @ q) + x

The row-max shift of the reference softmax cancels algebraically; only the
row minimum is needed for numerical stability (arguments of exp stay <= 0).

Key structure (v2 — XBAR-transpose restructure):
 - q^T is produced by the DMA XBAR transpose engine (dma_start_transpose),
   viewing adjacent fp8 pairs as fp16.  This removes all 128 identity-matmul
   PE transposes and the 32 DVE PSUM->SBUF copies of the previous version.
   The XBAR output interleaves the two fp8 values of each pair along the
   free dim (A0 B0 A1 B1 ...), which is exactly the DoubleRowSwInterleave
   weight layout (A/B pairs per column, columns reversed), so the energy
   matmuls consume it directly:
     * lhsT = raw interleaved bytes of one 128-c block -> output rows come
       out REVERSED within each 128 block (c_local = 127 - p).
     * rhs  = byte-strided [p, r, (ci c)] view of the same tiles.
 - All row-wise softmax steps run unchanged on the reversed rows.  The
   reversal is undone for free in the attT transposes by using a FLIP
   (anti-identity) matrix instead of the identity as the moving operand;
   the per-row scale gamma/Z is un-reversed by one tiny fp32 matmul
   against a fp32 flip matrix (b2f = J @ bias2).
 - x loads are raw pre-tile DMAs with manual completion semaphores (the
   tile scheduler caps outstanding in-tile DMAs with a small sliding
   window, which would serialize loads behind XBAR completions).  Sync
   issues gg0-1 after clearing their sems itself; GPSIMD issues gg2-3
   after the general semaphore clear.  All 32 half-casts run on DVE
   (~426ns each), gated on the load sems post-scheduling; ACT issues the
   gg0 XBARs, sync the rest.
 - Phase C: att@q DoubleRow matmuls into [P,1024] 2-bank PSUM groups; the
   epilogue out = po*(gamma/Z) + x is drained by 1024-col ops balanced
   across DVE (scalar_tensor_tensor) and ACT-mul + GPSIMD-add pairs, fp16
   store in 8 x 1MB-ish DMAs.
 - PSUM: 4 banks energy accumulators (reused in phase C for the attT
   staging and the bias2 flip), 4 banks for the [P,1024] att@q groups.
"""

import numpy as np

import concourse.bass as bass
import concourse.tile as tile
from concourse import mybir
from concourse.vector_clock import ScopedClock

P = 128
C = 512
N = 4096
B = 8
CT = C // P   # 4 c-tiles
GG = 4        # 1024-col load groups
UT = N // 256  # 16 uchunks (256 n-values each)

STRIP_TAIL = True

FP32 = mybir.dt.float32
FP16 = mybir.dt.float16
FP8 = mybir.dt.float8e4
DR = mybir.MatmulPerfMode.DoubleRow
DRI = mybir.MatmulPerfMode.DoubleRowSwInterleave
MIN = mybir.AluOpType.min


def _drain_and_barrier_split(self, tick_clock, wait_clock):
    # The pinned walrus rejects >1 sync-wait on TPB_CTRL (Drain); spread the
    # final global-clock waits across a chain of drains, one wait each.
    nc = self.nc
    drain_inst = nc.sync.drain()
    wait_clock.add_sem_waits(
        drain_inst.ins, ScopedClock({None: tick_clock.global_clock})
    )
    si = drain_inst.ins.sync_info
    if si is not None and si.on_wait is not None and len(si.on_wait) > 1:
        waits = list(si.on_wait)
        si.on_wait = waits[:1]
        for w in waits[1:]:
            extra = nc.sync.drain()
            extra.ins.sync_info = mybir.SyncInfo(on_wait=[w], on_update=[])
    nc.all_engine_barrier()
    assert self.sems is not None
    popped = nc._tile_sem_poison_stack.pop()
    assert popped is self._sem_poison
    if not STRIP_TAIL:
        nc.clear_and_free_semaphores(list(self.sems.allocated().values()))
        nc.all_engine_barrier()


tile.TileContext._drain_and_barrier = _drain_and_barrier_split


def _legalize_sync_waits(nc):
    # This walrus build rejects instructions carrying more than one sync-wait.
    # Hoist extra waits onto same-engine NoOps placed immediately before the
    # instruction (engine streams preserve relative order within a block).
    for f in nc.m.functions:
        for bb in f.blocks:
            new = []
            for inst in bb.instructions:
                si = inst.sync_info
                if si is not None and si.on_wait and len(si.on_wait) > 1:
                    waits = list(si.on_wait)
                    for w in waits[:-1]:
                        nop = mybir.InstNoOp(
                            name=nc.get_next_instruction_name(),
                            engine=inst.engine,
                            bass_nofuse=True,
                            sync_info=mybir.SyncInfo(on_wait=[w], on_update=[]),
                        )
                        new.append(nop)
                    si.on_wait = [waits[-1]]
                new.append(inst)
            bb.instructions[:] = new


def make_flip(nc, out, sq=P):
    # anti-identity: out[x, y] = 1 iff x + y == sq-1
    nc.gpsimd.memset(out, 0.0)
    nc.gpsimd.affine_select(
        out=out, in_=out,
        compare_op=mybir.AluOpType.not_equal,
        fill=1.0,
        base=-(sq - 1),
        pattern=[[1, sq]],
        channel_multiplier=1,
    )


def build_nc():
    nc = bass.Bass()
    x_d = nc.declare_dram_parameter("x", [C, N], FP32, isOutput=False)
    g_d = nc.declare_dram_parameter("gamma", [1, 1], FP32, isOutput=False)
    o_d = nc.declare_dram_parameter("out", [C, N], FP16, isOutput=True)

    # x-load completion semaphores, one per (gg, ci) chunk, plus one for the
    # gamma broadcast.
    xsem = [[nc.alloc_semaphore(f"xld{gg}_{ci}") for ci in range(CT)] for gg in range(GG)]
    gsem = nc.alloc_semaphore("gld")
    xf = [nc.alloc_sbuf_tensor(f"xraw{ci}", [P, N], FP32) for ci in range(CT)]
    gcol_raw = nc.alloc_sbuf_tensor("gcolraw", [P, 1], FP32)

    # Clear kernel semaphores at START (idle window) instead of paying the
    # expensive teardown clear+barrier at the end (STRIP_TAIL above).
    # (Dropping this hangs the device — semaphore state persists across NEFF
    # loads.)  All raw loads are issued AFTER the pseudo barrier so no engine
    # waits on another's issue backlog: sync takes gg0-1, gpsimd gg2-3 +
    # gamma.  In-tile consumers get explicit sem waits after scheduling.
    from concourse.bass import compact_to_ranges

    for sem_range in compact_to_ranges(
        [sem for sem in nc._kernel_sem_range if sem not in nc.barrier_sems]
    ):
        nc.gpsimd.sem_clear(sem_range)
    nc._nrt_pseudo_barrier()
    # One queue, consumption order: a single software-DGE queue processes its
    # transfers strictly in order at full per-transfer bandwidth, so chunk
    # (gg, ci) lands every ~1.3us exactly when the cast stream wants it.
    # Splitting across queues makes all transfers proceed concurrently and
    # collapses the arrival pacing.
    nc.gpsimd.dma_start(
        out=gcol_raw[:, :], in_=g_d[:, :].to_broadcast((P, 1))
    ).then_inc(gsem, 16)
    for gg in range(GG):
        for ci in range(CT):
            nc.gpsimd.dma_start(
                out=xf[ci][:, gg * 1024:(gg + 1) * 1024],
                in_=x_d[ci * P:(ci + 1) * P, gg * 1024:(gg + 1) * 1024],
            ).then_inc(xsem[gg][ci], 16)

    cast_waits = []  # (BassInstruction, gg, ci): xsem waits attached post-scheduling
    gcol_waits = []  # instructions reading gcol_raw: gsem waits attached post-scheduling

    with tile.TileContext(nc) as tc:
        with (
            tc.tile_pool(name="singles", bufs=1) as singles,
            tc.tile_pool(name="stage", bufs=4) as stage,
            tc.tile_pool(name="psum_acc", bufs=4, space="PSUM") as psum_acc,
            tc.tile_pool(name="psum_po", bufs=3, space="PSUM") as psum_po,
            tc.tile_pool(name="psum_flip", bufs=1, space="PSUM") as psum_flip,
        ):
            # Constants on GPSIMD (free early); PE warm-up on a dep-free
            # tile; ACT Exp-table preload on a dummy.
            warm8 = singles.tile([P, P], FP8, tag="warm8")
            nc.vector.memset(warm8[:], 1.0)
            J8 = singles.tile([P, P], FP8, tag="J8")
            make_flip(nc, J8[:])
            J32 = singles.tile([P, P], FP32, tag="J32")
            make_flip(nc, J32[:])
            gcol = singles.tile([P, 1], FP32, tag="gamma")

            for _ in range(12):
                wp = psum_po.tile([P, 512], FP32, tag="po")
                nc.tensor.matmul(
                    wp[:, 0:P], lhsT=warm8[:], rhs=warm8[:], start=True, stop=True
                )
            dume = singles.tile([P, 1], FP32, tag="dume")
            nc.scalar.activation(
                out=dume[:], in_=warm8[:, 0:1], func=mybir.ActivationFunctionType.Exp
            )

            q8 = singles.tile([P, CT, N], FP8, tag="q8")
            qTp = singles.tile([P, CT, UT, P], FP16, tag="qTp")
            e_ps = [
                psum_acc.tile([P, C], FP32, tag="acc", name=f"e{ci}")
                for ci in range(CT)
            ]

            # Phase A per 1024-col group gg:
            #  - fp32->fp8 casts both halves on DVE, gated on the raw load
            #    sems (attached after tile scheduling)
            #  - XBAR transpose per (gg, ci): fp8 pairs viewed as fp16,
            #    [128, 512]f16 -> [128, 4, 128] into qTp[:, ci, gg*4:gg*4+4, :]
            #    (ACT issues gg0 — sync is still busy issuing the loads)
            #  - 16 energy DRI matmuls per gg (4 uchunks x 4 ci rows)
            for gg in range(GG):
                lo = gg * 1024
                for ci in range(CT):
                    for half in range(2):
                        cst = nc.vector.tensor_copy(
                            out=q8[:, ci, lo + half * 512:lo + (half + 1) * 512],
                            in_=xf[ci][:, lo + half * 512:lo + (half + 1) * 512],
                        )
                        cast_waits.append((cst, gg, ci))
                for ci in range(CT):
                    nc.scalar.dma_start_transpose(
                        out=qTp[:, ci, gg * 4:(gg + 1) * 4, :],
                        in_=q8[:, ci, lo:lo + 1024].bitcast(FP16),
                    )
                for tt in range(4):
                    t = gg * 4 + tt
                    rhs = qTp[:, :, t, :].bitcast(FP8).rearrange(
                        "p ci (c r) -> p r ci c", r=2
                    )
                    for ci in range(CT):
                        nc.tensor.matmul(
                            e_ps[ci][:],
                            lhsT=qTp[:, ci, t, :].bitcast(FP8),
                            rhs=rhs,
                            start=(t == 0),
                            stop=(t == UT - 1),
                            perf_mode=DRI,
                        )

            # gamma broadcast into a tracked tile (emitted after the casts so
            # it never blocks the DVE cast stream; the raw DMA landed long
            # ago).
            gcp = nc.vector.tensor_copy(out=gcol[:], in_=gcol_raw[:, :])
            gcol_waits.append(gcp)

            # Softmax per ci on the REVERSED rows: row min (DVE), exp with
            # fp8 out + row-sum accumulator (ACT), 1/Z and gamma/Z (DVE
            # small).  attT via fp8 matmuls against the FLIP matrix (undoes
            # the row reversal), ACT strided copy into EXPT.  ci=0 runs its
            # min/exp in halves to shorten the serial chain into att@q.
            mcol = singles.tile([P, CT], FP32, tag="m")
            mh = singles.tile([P, 2], FP32, tag="mh")
            zcol = singles.tile([P, CT], FP32, tag="z")
            zh = singles.tile([P, 2], FP32, tag="zh")
            lnz = singles.tile([P, CT], FP32, tag="lnz")
            bias2 = singles.tile([P, CT], FP32, tag="bias2")
            b2f = singles.tile([P, CT], FP32, tag="b2f")
            EXPQ = singles.tile([P, CT, C], FP8, tag="EXPQ")
            EXPT = singles.tile([P, CT, C], FP8, tag="EXPT")

            def softmax_head(ci):
                cs = slice(ci, ci + 1)
                if ci == 0:
                    nc.vector.tensor_reduce(
                        out=mh[:, 0:1], in_=e_ps[0][:, 0:256],
                        axis=mybir.AxisListType.X, op=MIN,
                    )
                    nc.vector.tensor_reduce(
                        out=mh[:, 1:2], in_=e_ps[0][:, 256:512],
                        axis=mybir.AxisListType.X, op=MIN,
                    )
                    nc.vector.tensor_tensor(
                        out=mcol[:, 0:1], in0=mh[:, 0:1], in1=mh[:, 1:2], op=MIN
                    )
                else:
                    nc.vector.tensor_reduce(
                        out=mcol[:, cs], in_=e_ps[ci][:],
                        axis=mybir.AxisListType.X, op=MIN,
                    )

            # b2ps: single [P, CT] PSUM tile (from the freed-energy-bank
            # ring) collecting the flipped bias2 columns as they appear.
            b2ps_holder = []

            def softmax_tail(ci):
                cs = slice(ci, ci + 1)
                if ci == 0:
                    for half in range(2):
                        nc.scalar.activation(
                            out=EXPQ[:, 0, half * 256:(half + 1) * 256],
                            in_=e_ps[0][:, half * 256:(half + 1) * 256],
                            func=mybir.ActivationFunctionType.Exp,
                            bias=mcol[:, 0:1],
                            scale=-1.0,
                            accum_out=zh[:, half:half + 1],
                        )
                    nc.vector.tensor_add(
                        out=zcol[:, 0:1], in0=zh[:, 0:1], in1=zh[:, 1:2]
                    )
                else:
                    nc.scalar.activation(
                        out=EXPQ[:, ci, :],
                        in_=e_ps[ci][:],
                        func=mybir.ActivationFunctionType.Exp,
                        bias=mcol[:, cs],
                        scale=-1.0,
                        accum_out=zcol[:, cs],
                    )
                nc.vector.reciprocal(out=lnz[:, cs], in_=zcol[:, cs])
                nc.vector.tensor_mul(out=bias2[:, cs], in0=lnz[:, cs], in1=gcol[:])
                # un-reverse bias2 rows: b2f[p] = bias2[127-p] (per column)
                if not b2ps_holder:
                    b2ps_holder.append(
                        psum_flip.tile([P, CT], FP32, tag="b2ps", name="b2ps")
                    )
                b2ps = b2ps_holder[0]
                nc.tensor.matmul(
                    b2ps[:, cs], lhsT=J32[:], rhs=bias2[:, cs],
                    start=True, stop=True,
                )
                nc.vector.tensor_copy(out=b2f[:, cs], in_=b2ps[:, cs])
                # attT transposes vs the flip matrix: EXPT columns come out
                # with FORWARD c.  All four land in quarters of one PSUM
                # bank, then one strided ACT copy scatters them into EXPT.
                ptx = psum_acc.tile([P, C], FP32, tag="acc", name=f"ptx{ci}")
                for dj in range(CT):
                    nc.tensor.matmul(
                        ptx[:, dj * P:(dj + 1) * P],
                        lhsT=EXPQ[:, ci, dj * P:(dj + 1) * P],
                        rhs=J8[:],
                        start=True,
                        stop=True,
                    )
                nc.scalar.copy(
                    out=EXPT[:, :, ci * P:(ci + 1) * P],
                    in_=ptx[:].rearrange("p (d j) -> p d j", d=CT),
                )

            def attq(ci):
                # att@q (DoubleRow, K=512 via dj pairs) + fused epilogue add
                # out = po * (gamma/Z_c) + x, fp16 store.  3 subs drain via
                # DVE scalar_tensor_tensor, the 4th via ACT-scale +
                # GPSIMD-add; sub 3 reuses a freed energy-accumulator bank
                # so the matmuls never stall on the adds.
                for nh in range(2):
                    osb = stage.tile([P, 2048], FP16, tag="osb")
                    for sub in range(4):
                        nj = nh * 4 + sub
                        if sub == 3:
                            po = psum_acc.tile([P, 512], FP32, tag="acc", name="po")
                        else:
                            po = psum_po.tile([P, 512], FP32, tag="po")
                        for j in range(2):
                            nc.tensor.matmul(
                                po[:],
                                lhsT=EXPT[:, 2 * j:2 * j + 2, ci * P:(ci + 1) * P],
                                rhs=q8[:, 2 * j:2 * j + 2, nj * 512:(nj + 1) * 512],
                                start=(j == 0),
                                stop=(j == 1),
                                perf_mode=DR,
                            )
                        if sub < 3:
                            nc.vector.scalar_tensor_tensor(
                                out=osb[:, sub * 512:(sub + 1) * 512],
                                in0=po[:],
                                scalar=b2f[:, ci:ci + 1],
                                in1=xf[ci][:, nj * 512:(nj + 1) * 512],
                                op0=mybir.AluOpType.mult,
                                op1=mybir.AluOpType.add,
                            )
                        else:
                            tmp = stage.tile([P, 512], FP32, tag="tmp")
                            nc.scalar.mul(
                                out=tmp[:], in_=po[:], mul=b2f[:, ci:ci + 1]
                            )
                            nc.gpsimd.tensor_add(
                                out=osb[:, sub * 512:(sub + 1) * 512],
                                in0=tmp[:],
                                in1=xf[ci][:, nj * 512:(nj + 1) * 512],
                            )
                    nc.sync.dma_start(
                        out=o_d[ci * P:(ci + 1) * P, nh * 2048:(nh + 1) * 2048],
                        in_=osb[:],
                    )

            for ci in range(CT):
                softmax_head(ci)
            for ci in range(CT):
                softmax_tail(ci)
                attq(ci)

    # The raw-load gating is invisible to the tile scheduler (its deadlock
    # simulator would stall on semaphores no in-context instruction bumps),
    # so attach the waits only after scheduling has run.
    for cst, gg, ci in cast_waits:
        cst.wait_op(xsem[gg][ci], 16, "sem-ge")
    for ins in gcol_waits:
        ins.wait_op(gsem, 16, "sem-ge")
    _legalize_sync_waits(nc)
    return nc


def make_in_maps(x, gamma):
    x = np.ascontiguousarray(np.asarray(x, dtype=np.float32)).reshape(B, C, N)
    g = np.ascontiguousarray(np.asarray(gamma, dtype=np.float32)).reshape(1, 1)
    return [{"x": x[i], "gamma": g} for i in range(B)]


def kernel(x, y=None, gamma=None, **_ignored):
    from concourse.bass_utils import run_bass_kernel_spmd

    nc = build_nc()
    in_maps = make_in_maps(x, gamma)
    res = run_bass_kernel_spmd(nc, in_maps, list(range(B)))
    out = np.stack([np.asarray(res.results[i]["out"]) for i in range(B)])
    return out.reshape(B, C, 64, 64).astype(np.float32)


# revision 27
# speedup vs baseline: 1.8145x; 1.0293x over previous
"""CAM-module kernel for Trainium2, data-parallel over batch on 8 NeuronCores.

Per core (one batch sample, q = x[b] viewed as (C=512, N=4096) fp32):
  energy   = q @ q^T                      (C, C)   fp8 matmul, fp32 accum
  att[c,d] = exp(m_c - e[c,d]) / Z_c      with m_c = row min of energy
  out      = gamma * (att @ q) + x

The row-max shift of the reference softmax cancels algebraically; only the
row minimum is needed for numerical stability (arguments of exp stay <= 0).

Key structure (v2 — XBAR-transpose restructure):
 - q^T is produced by the DMA XBAR transpose engine (dma_start_transpose),
   viewing adjacent fp8 pairs as fp16.  This removes all 128 identity-matmul
   PE transposes and the 32 DVE PSUM->SBUF copies of the previous version.
   The XBAR output interleaves the two fp8 values of each pair along the
   free dim (A0 B0 A1 B1 ...), which is exactly the DoubleRowSwInterleave
   weight layout (A/B pairs per column, columns reversed), so the energy
   matmuls consume it directly:
     * lhsT = raw interleaved bytes of one 128-c block -> output rows come
       out REVERSED within each 128 block (c_local = 127 - p).
     * rhs  = byte-strided [p, r, (ci c)] view of the same tiles.
 - All row-wise softmax steps run unchanged on the reversed rows.  The
   reversal is undone for free in the attT transposes by using a FLIP
   (anti-identity) matrix instead of the identity as the moving operand;
   the per-row scale gamma/Z is un-reversed by one tiny fp32 matmul
   against a fp32 flip matrix (b2f = J @ bias2).
 - x loads are raw pre-tile DMAs with manual completion semaphores (the
   tile scheduler caps outstanding in-tile DMAs with a small sliding
   window, which would serialize loads behind XBAR completions).  Sync
   issues gg0-1 after clearing their sems itself; GPSIMD issues gg2-3
   after the general semaphore clear.  All 32 half-casts run on DVE
   (~426ns each), gated on the load sems post-scheduling; ACT issues the
   gg0 XBARs, sync the rest.
 - Phase C: att@q DoubleRow matmuls into [P,1024] 2-bank PSUM groups; the
   epilogue out = po*(gamma/Z) + x is drained by 1024-col ops balanced
   across DVE (scalar_tensor_tensor) and ACT-mul + GPSIMD-add pairs, fp16
   store in 8 x 1MB-ish DMAs.
 - PSUM: 4 banks energy accumulators (reused in phase C for the attT
   staging and the bias2 flip), 4 banks for the [P,1024] att@q groups.
"""

import numpy as np

import concourse.bass as bass
import concourse.tile as tile
from concourse import mybir
from concourse.vector_clock import ScopedClock

P = 128
C = 512
N = 4096
B = 8
CT = C // P   # 4 c-tiles
GG = 4        # 1024-col load groups
UT = N // 256  # 16 uchunks (256 n-values each)

STRIP_TAIL = True

FP32 = mybir.dt.float32
FP16 = mybir.dt.float16
FP8 = mybir.dt.float8e4
DR = mybir.MatmulPerfMode.DoubleRow
DRI = mybir.MatmulPerfMode.DoubleRowSwInterleave
MIN = mybir.AluOpType.min


def _drain_and_barrier_split(self, tick_clock, wait_clock):
    # The pinned walrus rejects >1 sync-wait on TPB_CTRL (Drain); spread the
    # final global-clock waits across a chain of drains, one wait each.
    nc = self.nc
    drain_inst = nc.sync.drain()
    wait_clock.add_sem_waits(
        drain_inst.ins, ScopedClock({None: tick_clock.global_clock})
    )
    si = drain_inst.ins.sync_info
    if si is not None and si.on_wait is not None and len(si.on_wait) > 1:
        waits = list(si.on_wait)
        si.on_wait = waits[:1]
        for w in waits[1:]:
            extra = nc.sync.drain()
            extra.ins.sync_info = mybir.SyncInfo(on_wait=[w], on_update=[])
    nc.all_engine_barrier()
    assert self.sems is not None
    popped = nc._tile_sem_poison_stack.pop()
    assert popped is self._sem_poison
    if not STRIP_TAIL:
        nc.clear_and_free_semaphores(list(self.sems.allocated().values()))
        nc.all_engine_barrier()


tile.TileContext._drain_and_barrier = _drain_and_barrier_split


def _legalize_sync_waits(nc):
    # This walrus build rejects instructions carrying more than one sync-wait.
    # Hoist extra waits onto same-engine NoOps placed immediately before the
    # instruction (engine streams preserve relative order within a block).
    for f in nc.m.functions:
        for bb in f.blocks:
            new = []
            for inst in bb.instructions:
                si = inst.sync_info
                if si is not None and si.on_wait and len(si.on_wait) > 1:
                    waits = list(si.on_wait)
                    for w in waits[:-1]:
                        nop = mybir.InstNoOp(
                            name=nc.get_next_instruction_name(),
                            engine=inst.engine,
                            bass_nofuse=True,
                            sync_info=mybir.SyncInfo(on_wait=[w], on_update=[]),
                        )
                        new.append(nop)
                    si.on_wait = [waits[-1]]
                new.append(inst)
            bb.instructions[:] = new


def make_flip(nc, out, sq=P):
    # anti-identity: out[x, y] = 1 iff x + y == sq-1
    nc.gpsimd.memset(out, 0.0)
    nc.gpsimd.affine_select(
        out=out, in_=out,
        compare_op=mybir.AluOpType.not_equal,
        fill=1.0,
        base=-(sq - 1),
        pattern=[[1, sq]],
        channel_multiplier=1,
    )


def build_nc():
    nc = bass.Bass()
    x_d = nc.declare_dram_parameter("x", [C, N], FP32, isOutput=False)
    g_d = nc.declare_dram_parameter("gamma", [1, 1], FP32, isOutput=False)
    o_d = nc.declare_dram_parameter("out", [C, N], FP16, isOutput=True)

    # x-load completion semaphores, one per (gg, ci) chunk, plus one for the
    # gamma broadcast.
    xsem = [[nc.alloc_semaphore(f"xld{gg}_{ci}") for ci in range(CT)] for gg in range(GG)]
    gsem = nc.alloc_semaphore("gld")
    xf = [nc.alloc_sbuf_tensor(f"xraw{ci}", [P, N], FP32) for ci in range(CT)]
    gcol_raw = nc.alloc_sbuf_tensor("gcolraw", [P, 1], FP32)

    # Clear kernel semaphores at START (idle window) instead of paying the
    # expensive teardown clear+barrier at the end (STRIP_TAIL above).
    # (Dropping this hangs the device — semaphore state persists across NEFF
    # loads.)  All raw loads are issued AFTER the pseudo barrier so no engine
    # waits on another's issue backlog: sync takes gg0-1, gpsimd gg2-3 +
    # gamma.  In-tile consumers get explicit sem waits after scheduling.
    from concourse.bass import compact_to_ranges

    for sem_range in compact_to_ranges(
        [sem for sem in nc._kernel_sem_range if sem not in nc.barrier_sems]
    ):
        nc.gpsimd.sem_clear(sem_range)
    nc._nrt_pseudo_barrier()
    # One queue, consumption order: a single software-DGE queue processes its
    # transfers strictly in order at full per-transfer bandwidth, so chunk
    # (gg, ci) lands every ~1.3us exactly when the cast stream wants it.
    # Splitting across queues makes all transfers proceed concurrently and
    # collapses the arrival pacing.
    for gg in range(GG):
        for ci in range(CT):
            nc.gpsimd.dma_start(
                out=xf[ci][:, gg * 1024:(gg + 1) * 1024],
                in_=x_d[ci * P:(ci + 1) * P, gg * 1024:(gg + 1) * 1024],
            ).then_inc(xsem[gg][ci], 16)
    # gamma broadcast LAST: its 128 tiny descriptors take ~3.5us of queue
    # head time and it is not needed until the softmax.
    nc.gpsimd.dma_start(
        out=gcol_raw[:, :], in_=g_d[:, :].to_broadcast((P, 1))
    ).then_inc(gsem, 16)

    cast_waits = []  # (BassInstruction, gg, ci): xsem waits attached post-scheduling
    gcol_waits = []  # instructions reading gcol_raw: gsem waits attached post-scheduling

    with tile.TileContext(nc) as tc:
        with (
            tc.tile_pool(name="singles", bufs=1) as singles,
            tc.tile_pool(name="stage", bufs=4) as stage,
            tc.tile_pool(name="psum_acc", bufs=4, space="PSUM") as psum_acc,
            tc.tile_pool(name="psum_po", bufs=3, space="PSUM") as psum_po,
            tc.tile_pool(name="psum_flip", bufs=1, space="PSUM") as psum_flip,
        ):
            # Constants on GPSIMD (free early); PE warm-up on a dep-free
            # tile; ACT Exp-table preload on a dummy.
            warm8 = singles.tile([P, P], FP8, tag="warm8")
            nc.vector.memset(warm8[:], 1.0)
            J8 = singles.tile([P, P], FP8, tag="J8")
            make_flip(nc, J8[:])
            J32 = singles.tile([P, P], FP32, tag="J32")
            make_flip(nc, J32[:])
            gcol = singles.tile([P, 1], FP32, tag="gamma")

            for _ in range(12):
                wp = psum_po.tile([P, 512], FP32, tag="po")
                nc.tensor.matmul(
                    wp[:, 0:P], lhsT=warm8[:], rhs=warm8[:], start=True, stop=True
                )
            dume = singles.tile([P, 1], FP32, tag="dume")
            nc.scalar.activation(
                out=dume[:], in_=warm8[:, 0:1], func=mybir.ActivationFunctionType.Exp
            )

            q8 = singles.tile([P, CT, N], FP8, tag="q8")
            # qTp layout [P, gg, ci, t-within-gg, c]: the (gg, ci) XBAR
            # destination is one contiguous 1KB block per partition, AND the
            # energy rhs read [:, gg, :, tw, :] stays inside gg's 4KB block —
            # the tile tracker's coarse byte-range dependency analysis then
            # links each energy matmul only to its own gg's XBARs (with ci as
            # an outer dim the rhs range spanned the whole tile and picked up
            # false deps on later groups' transposes).
            qTp = singles.tile([P, GG, CT, 4, P], FP16, tag="qTp")
            e_ps = [
                psum_acc.tile([P, C], FP32, tag="acc", name=f"e{ci}")
                for ci in range(CT)
            ]

            # Phase A per 1024-col group gg:
            #  - fp32->fp8 casts both halves on DVE, gated on the raw load
            #    sems (attached after tile scheduling)
            #  - XBAR transpose per (gg, ci): fp8 pairs viewed as fp16,
            #    [128, 512]f16 -> [128, 4, 128] into qTp[:, gg, ci]; the
            #    transpose blocks its issuing engine for the whole transfer,
            #    so they are split sync/ACT two each per gg
            #  - 16 energy DRI matmuls per gg (4 uchunks x 4 ci rows)
            for gg in range(GG):
                lo = gg * 1024
                for ci in range(CT):
                    for half in range(2):
                        cst = nc.vector.tensor_copy(
                            out=q8[:, ci, lo + half * 512:lo + (half + 1) * 512],
                            in_=xf[ci][:, lo + half * 512:lo + (half + 1) * 512],
                        )
                        cast_waits.append((cst, gg, ci))
                for ci in range(CT):
                    xbar_eng = nc.sync if ci < 2 else nc.scalar
                    xbar_eng.dma_start_transpose(
                        out=qTp[:, gg, ci, :, :],
                        in_=q8[:, ci, lo:lo + 1024].bitcast(FP16),
                    )
                for tw in range(4):
                    t = gg * 4 + tw
                    rhs = qTp[:, gg, :, tw, :].bitcast(FP8).rearrange(
                        "p ci (c r) -> p r ci c", r=2
                    )
                    for ci in range(CT):
                        nc.tensor.matmul(
                            e_ps[ci][:],
                            lhsT=qTp[:, gg, ci, tw, :].bitcast(FP8),
                            rhs=rhs,
                            start=(t == 0),
                            stop=(t == UT - 1),
                            perf_mode=DRI,
                        )

            # gamma broadcast into a tracked tile (emitted after the casts so
            # it never blocks the DVE cast stream; the raw DMA landed long
            # ago).
            gcp = nc.vector.tensor_copy(out=gcol[:], in_=gcol_raw[:, :])
            gcol_waits.append(gcp)

            # Softmax per ci on the REVERSED rows: row min (DVE), exp with
            # fp8 out + row-sum accumulator (ACT), 1/Z and gamma/Z (DVE
            # small).  attT via fp8 matmuls against the FLIP matrix (undoes
            # the row reversal), ACT strided copy into EXPT.  ci=0 runs its
            # min/exp in halves to shorten the serial chain into att@q.
            mcol = singles.tile([P, CT], FP32, tag="m")
            mh = singles.tile([P, 2], FP32, tag="mh")
            zcol = singles.tile([P, CT], FP32, tag="z")
            zh = singles.tile([P, 2], FP32, tag="zh")
            lnz = singles.tile([P, CT], FP32, tag="lnz")
            bias2 = singles.tile([P, CT], FP32, tag="bias2")
            b2f = singles.tile([P, CT], FP32, tag="b2f")
            EXPQ = singles.tile([P, CT, C], FP8, tag="EXPQ")
            EXPT = singles.tile([P, CT, C], FP8, tag="EXPT")

            def softmax_head(ci):
                cs = slice(ci, ci + 1)
                if ci == 0:
                    nc.vector.tensor_reduce(
                        out=mh[:, 0:1], in_=e_ps[0][:, 0:256],
                        axis=mybir.AxisListType.X, op=MIN,
                    )
                    nc.vector.tensor_reduce(
                        out=mh[:, 1:2], in_=e_ps[0][:, 256:512],
                        axis=mybir.AxisListType.X, op=MIN,
                    )
                    nc.vector.tensor_tensor(
                        out=mcol[:, 0:1], in0=mh[:, 0:1], in1=mh[:, 1:2], op=MIN
                    )
                else:
                    nc.vector.tensor_reduce(
                        out=mcol[:, cs], in_=e_ps[ci][:],
                        axis=mybir.AxisListType.X, op=MIN,
                    )

            # b2ps: single [P, CT] PSUM tile (from the freed-energy-bank
            # ring) collecting the flipped bias2 columns as they appear.
            b2ps_holder = []

            def softmax_tail(ci):
                cs = slice(ci, ci + 1)
                if ci == 0:
                    for half in range(2):
                        nc.scalar.activation(
                            out=EXPQ[:, 0, half * 256:(half + 1) * 256],
                            in_=e_ps[0][:, half * 256:(half + 1) * 256],
                            func=mybir.ActivationFunctionType.Exp,
                            bias=mcol[:, 0:1],
                            scale=-1.0,
                            accum_out=zh[:, half:half + 1],
                        )
                    nc.vector.tensor_add(
                        out=zcol[:, 0:1], in0=zh[:, 0:1], in1=zh[:, 1:2]
                    )
                else:
                    nc.scalar.activation(
                        out=EXPQ[:, ci, :],
                        in_=e_ps[ci][:],
                        func=mybir.ActivationFunctionType.Exp,
                        bias=mcol[:, cs],
                        scale=-1.0,
                        accum_out=zcol[:, cs],
                    )
                nc.vector.reciprocal(out=lnz[:, cs], in_=zcol[:, cs])
                nc.vector.tensor_mul(out=bias2[:, cs], in0=lnz[:, cs], in1=gcol[:])
                # un-reverse bias2 rows: b2f[p] = bias2[127-p] (per column)
                if not b2ps_holder:
                    b2ps_holder.append(
                        psum_flip.tile([P, CT], FP32, tag="b2ps", name="b2ps")
                    )
                b2ps = b2ps_holder[0]
                nc.tensor.matmul(
                    b2ps[:, cs], lhsT=J32[:], rhs=bias2[:, cs],
                    start=True, stop=True,
                )
                nc.vector.tensor_copy(out=b2f[:, cs], in_=b2ps[:, cs])
                # attT transposes vs the flip matrix: EXPT columns come out
                # with FORWARD c.  All four land in quarters of one PSUM
                # bank, then one strided ACT copy scatters them into EXPT.
                ptx = psum_acc.tile([P, C], FP32, tag="acc", name=f"ptx{ci}")
                for dj in range(CT):
                    nc.tensor.matmul(
                        ptx[:, dj * P:(dj + 1) * P],
                        lhsT=EXPQ[:, ci, dj * P:(dj + 1) * P],
                        rhs=J8[:],
                        start=True,
                        stop=True,
                    )
                nc.scalar.copy(
                    out=EXPT[:, :, ci * P:(ci + 1) * P],
                    in_=ptx[:].rearrange("p (d j) -> p d j", d=CT),
                )

            def attq(ci):
                # att@q (DoubleRow, K=512 via dj pairs) + fused epilogue add
                # out = po * (gamma/Z_c) + x, fp16 store.  3 subs drain via
                # DVE scalar_tensor_tensor, the 4th via ACT-scale +
                # GPSIMD-add; sub 3 reuses a freed energy-accumulator bank
                # so the matmuls never stall on the adds.
                for nh in range(2):
                    osb = stage.tile([P, 2048], FP16, tag="osb")
                    for sub in range(4):
                        nj = nh * 4 + sub
                        if sub == 3:
                            po = psum_acc.tile([P, 512], FP32, tag="acc", name="po")
                        else:
                            po = psum_po.tile([P, 512], FP32, tag="po")
                        for j in range(2):
                            nc.tensor.matmul(
                                po[:],
                                lhsT=EXPT[:, 2 * j:2 * j + 2, ci * P:(ci + 1) * P],
                                rhs=q8[:, 2 * j:2 * j + 2, nj * 512:(nj + 1) * 512],
                                start=(j == 0),
                                stop=(j == 1),
                                perf_mode=DR,
                            )
                        if sub < 3:
                            nc.vector.scalar_tensor_tensor(
                                out=osb[:, sub * 512:(sub + 1) * 512],
                                in0=po[:],
                                scalar=b2f[:, ci:ci + 1],
                                in1=xf[ci][:, nj * 512:(nj + 1) * 512],
                                op0=mybir.AluOpType.mult,
                                op1=mybir.AluOpType.add,
                            )
                        else:
                            tmp = stage.tile([P, 512], FP32, tag="tmp")
                            nc.scalar.mul(
                                out=tmp[:], in_=po[:], mul=b2f[:, ci:ci + 1]
                            )
                            nc.gpsimd.tensor_add(
                                out=osb[:, sub * 512:(sub + 1) * 512],
                                in0=tmp[:],
                                in1=xf[ci][:, nj * 512:(nj + 1) * 512],
                            )
                    nc.sync.dma_start(
                        out=o_d[ci * P:(ci + 1) * P, nh * 2048:(nh + 1) * 2048],
                        in_=osb[:],
                    )

            for ci in range(CT):
                softmax_head(ci)
            for ci in range(CT):
                softmax_tail(ci)
                attq(ci)

    # The raw-load gating is invisible to the tile scheduler (its deadlock
    # simulator would stall on semaphores no in-context instruction bumps),
    # so attach the waits only after scheduling has run.
    for cst, gg, ci in cast_waits:
        cst.wait_op(xsem[gg][ci], 16, "sem-ge")
    for ins in gcol_waits:
        ins.wait_op(gsem, 16, "sem-ge")
    _legalize_sync_waits(nc)
    return nc


def make_in_maps(x, gamma):
    x = np.ascontiguousarray(np.asarray(x, dtype=np.float32)).reshape(B, C, N)
    g = np.ascontiguousarray(np.asarray(gamma, dtype=np.float32)).reshape(1, 1)
    return [{"x": x[i], "gamma": g} for i in range(B)]


def kernel(x, y=None, gamma=None, **_ignored):
    from concourse.bass_utils import run_bass_kernel_spmd

    nc = build_nc()
    in_maps = make_in_maps(x, gamma)
    res = run_bass_kernel_spmd(nc, in_maps, list(range(B)))
    out = np.stack([np.asarray(res.results[i]["out"]) for i in range(B)])
    return out.reshape(B, C, 64, 64).astype(np.float32)
